# revision 4
# baseline (speedup 1.0000x reference)
"""DeflateVertexToHex Trainium2 kernel (bf16 transpose-gather pipeline).

out[b, t, :] = (mean over valid s of vertex_feats[b, hex_to_vertex[t, s], :]) @ W + b

Shapes: vertex_feats [4, 20000, 512] f32, hex_to_vertex [10000, 6] i64,
W [512, 512] f32, b [512] f32 -> out [4, 10000, 512] f32.

Sharding over 8 NeuronCores: core c handles batch c//2 and hex half c%2
(5000 hexes = 39 full 128-hex tiles + one 8-hex tail tile).

The kernel is HBM-gather bound, so all gathered bytes are bf16 (rel err
~4e-3 on the real inputs, well under the 2e-2 gate):
  1. The vertex table is converted host-side to bf16 [N+1, D] with an
     appended all-zero row that padded (-1) adjacency slots remap to.
  2. SWDGE dma_gather in TRANSPOSE mode pulls 3 tiles' worth of rows per
     instruction (2304 x 1KB) and lands them as columns: out[p, j, i] =
     row_i[j*128 + p], i.e. the [D, hex*slot] layout the matmul wants.
     This removes the 12 PE transposes + PSUM round-trip per tile that a
     row-major gather needs, and grouping 3 tiles per instruction cuts
     the 994 ns fixed SWDGE descriptor-generation cost to ~25 us total.
  3. DVE pools the 6 slots with 3 bf16 adds per tile (2-byte packed
     operands run in the 4x DVE perf mode).
  4. PE accumulates out[hex, H] over 4 K-chunk bf16 matmuls straight
     from the pooled SBUF tile (lhsT = pooled[:, c, :]). When the bias
     is nonzero an extra K=1 pass adds count[hex] * b so the final
     per-hex 1/count scale distributes over it correctly.
  5. ACT applies the per-hex 1/count scale PSUM->SBUF (per-partition
     scale operand); the 128x512 result is written to DRAM as bf16 and
     upcast host-side.
"""

import numpy as np
import ml_dtypes

import concourse.bacc as bacc
import concourse.tile as tile
from concourse import mybir
from concourse.bass_utils import run_bass_kernel_spmd

F32 = mybir.dt.float32
BF16 = mybir.dt.bfloat16
I16 = mybir.dt.int16

B = 4
N = 20000
D = 512
H = 512
T = 10000
S = 6
P = 128
N_CORES = 8
T_CORE = T // 2                  # 5000 hexes per core
FULL = T_CORE // P               # 39 full 128-hex tiles
LT_H = T_CORE - FULL * P         # 8 hexes in the tail tile
TILES = FULL + 1                 # 40
PADT = TILES * P                 # 5120
GROUP = 1                        # full tiles per dma_gather instruction
                                 # (transpose-mode SWDGE fails on HW between
                                 # 768 and 1536 idx; 768 = 1 tile works)
NGROUP = FULL // GROUP           # 13
NI = GROUP * S * P               # 2304 indices per grouped gather
IDXW = S * P // 16               # 48 idx columns per full tile
TAIL_NI = 128                    # tail gather indices (48 real, s*8+h)
IDX_COLS = FULL * IDXW + TAIL_NI // 16  # 1880
JD = D // P                      # 4 K-chunks


def _patch_tile_drain():
    """This container's walrus rejects >1 sync wait on the tail InstDrain
    emitted by TileContext; split the waits across single-wait SP nops."""
    if getattr(tile.TileContext, "_drain_patch_applied", False):
        return

    def _drain_and_barrier_split(self, tick_clock, wait_clock):
        nc = self.nc
        probe = nc.sync.nop(nofuse=True)
        wait_clock.add_sem_waits(
            probe.ins, tile.ScopedClock({None: tick_clock.global_clock})
        )
        si = probe.ins.sync_info
        waits = list(si.on_wait) if si is not None else []
        if si is not None:
            si.on_wait = []
        for w in waits[1:]:
            n = nc.sync.nop(nofuse=True)
            n.ins.sync_info = mybir.SyncInfo(on_wait=[w], on_update=[])
        if waits:
            probe.ins.sync_info = mybir.SyncInfo(on_wait=[waits[0]], on_update=[])
        nc.sync.drain()
        nc.all_engine_barrier()
        assert self.sems is not None
        popped = nc._tile_sem_poison_stack.pop()
        assert popped is self._sem_poison
        nc.clear_and_free_semaphores(list(self.sems.allocated().values()))
        nc.all_engine_barrier()

    tile.TileContext._drain_and_barrier = _drain_and_barrier_split
    tile.TileContext._drain_patch_applied = True


def build_module(include_bias=True, repeat=1, nq=4, scratch=65536, gbufs=2):
    _patch_tile_drain()
    nc = bacc.Bacc(
        "TRN2",
        target_bir_lowering=False,
        debug=False,
        num_swdge_queues=nq,
        dynamic_dma_scratch_size=scratch,
    )
    vtx = nc.declare_dram_parameter("vtx", [N + 1, D], BF16, isOutput=False)
    wm = nc.declare_dram_parameter("wmat", [D, H], BF16, isOutput=False)
    brow = nc.declare_dram_parameter("brow", [1, H], BF16, isOutput=False)
    cnt = nc.declare_dram_parameter("cnt", [1, PADT], BF16, isOutput=False)
    idx = nc.declare_dram_parameter("idx", [P, IDX_COLS], I16, isOutput=False)
    invc = nc.declare_dram_parameter("invc", [P, TILES], F32, isOutput=False)
    out = nc.declare_dram_parameter("out", [PADT, H], BF16, isOutput=True)

    with tile.TileContext(nc) as tc:
        with (
            tc.tile_pool(name="const", bufs=1) as constp,
            tc.tile_pool(name="gather", bufs=gbufs) as gpool,
            tc.tile_pool(name="tmp", bufs=2) as tmp,
            tc.tile_pool(name="pl", bufs=3) as plp,
            tc.tile_pool(name="osb", bufs=4) as osb,
            tc.tile_pool(name="mmps", bufs=3, space="PSUM") as mmps,
        ):
            # idx first: group 0's slice in its own small DMA so gather 0
            # isn't held behind the full index load.
            idx_sb = constp.tile([P, IDX_COLS], I16)
            c0 = GROUP * IDXW
            nc.sync.dma_start(idx_sb[:, :c0], idx[:, :c0])
            nc.sync.dma_start(idx_sb[:, c0:], idx[:, c0:])
            w_sb = constp.tile([P, JD * H], BF16)
            for c in range(JD):
                nc.sync.dma_start(
                    w_sb[:, c * H : (c + 1) * H], wm[c * P : (c + 1) * P, :]
                )
            invc_sb = constp.tile([P, TILES], F32)
            nc.sync.dma_start(invc_sb[:], invc[:])
            b_sb = constp.tile([1, H], BF16)
            nc.sync.dma_start(b_sb[:], brow[:])
            cnt_sb = constp.tile([1, PADT], BF16)
            nc.sync.dma_start(cnt_sb[:], cnt[:])

            import contextlib

            loop_ctx = tc.For_i(0, repeat, 1) if repeat > 1 else contextlib.nullcontext()
            with loop_ctx:
                for g in range(NGROUP + 1):
                    is_tail = g == NGROUP
                    ni = TAIL_NI if is_tail else NI
                    gt = gpool.tile([P, JD, ni], BF16, tag="gtail" if is_tail else "g")
                    col = g * GROUP * IDXW
                    nc.gpsimd.dma_gather(
                        gt[:],
                        vtx[:],
                        idx_sb[:, col : col + ni // 16],
                        ni,
                        ni,
                        D,
                        transpose=True,
                        queue_num=g % nq,
                    )
                    for k in range(1 if is_tail else GROUP):
                        t = g * GROUP + k
                        hexes = LT_H if is_tail else P
                        w3 = 3 * hexes          # three-slot block width
                        base = k * S * P
                        # slot pool: 2 x (s, s+3) pairs then fold 3 -> 1
                        q = tmp.tile([P, JD, w3], BF16, tag="q")
                        nc.vector.tensor_add(
                            q[:],
                            gt[:, :, base : base + w3],
                            gt[:, :, base + w3 : base + 2 * w3],
                        )
                        r = tmp.tile([P, JD, hexes], BF16, tag="r")
                        nc.vector.tensor_add(
                            r[:], q[:, :, 0:hexes], q[:, :, hexes : 2 * hexes]
                        )
                        pl = plp.tile([P, JD, hexes], BF16, tag="p")
                        nc.vector.tensor_add(pl[:], r[:], q[:, :, 2 * hexes : w3])
                        mmp = mmps.tile([P, H], F32)
                        for c in range(JD):
                            nc.tensor.matmul(
                                mmp[:hexes, :],
                                lhsT=pl[:, c, :],
                                rhs=w_sb[:, c * H : (c + 1) * H],
                                start=(c == 0),
                                stop=(c == JD - 1 and not include_bias),
                            )
                        if include_bias:
                            nc.tensor.matmul(
                                mmp[:hexes, :],
                                lhsT=cnt_sb[0:1, t * P : t * P + hexes],
                                rhs=b_sb[0:1, :],
                                start=False,
                                stop=True,
                            )
                        o = osb.tile([P, H], BF16, tag="o")
                        nc.scalar.mul(
                            o[:hexes, :], mmp[:hexes, :], invc_sb[:hexes, t : t + 1]
                        )
                        nc.sync.dma_start(out[t * P : t * P + hexes, :], o[:hexes, :])
    nc.finalize()
    return nc


def prep_inputs(vertex_feats, hex_to_vertex, W, b):
    """Host-side prep -> per-core in_maps."""
    vertex_feats = np.asarray(vertex_feats)
    hex_to_vertex = np.asarray(hex_to_vertex)
    W16 = np.ascontiguousarray(np.asarray(W).astype(ml_dtypes.bfloat16))
    b16 = np.asarray(b).astype(ml_dtypes.bfloat16).reshape(1, H)

    mask = hex_to_vertex >= 0
    safe = np.where(mask, hex_to_vertex, N).astype(np.int16)       # [T, 6]
    cntc = np.maximum(mask.sum(axis=1), 1).astype(np.float32)      # [T]
    inv = (1.0 / cntc).astype(np.float32)

    vtx_pads = []
    for bi in range(B):
        vp = np.zeros((N + 1, D), dtype=ml_dtypes.bfloat16)
        vp[:N] = vertex_feats[bi].astype(ml_dtypes.bfloat16)
        vtx_pads.append(vp)

    half_arrays = []
    for hh in range(2):
        sl = slice(hh * T_CORE, (hh + 1) * T_CORE)
        safe_h = safe[sl]                                          # [5000, 6]
        cnt_pad = np.ones(PADT, dtype=np.float32)
        cnt_pad[:T_CORE] = cntc[sl]
        inv_pad = np.ones(PADT, dtype=np.float32)
        inv_pad[:T_CORE] = inv[sl]
        # full tiles: flat[t, s*128 + p] = safe_h[t*128 + p, s]
        flat_full = (
            safe_h[: FULL * P]
            .reshape(FULL, P, S)
            .transpose(0, 2, 1)
            .reshape(FULL * S * P)
        )
        # tail: i = s*8 + h for s < 6, h < 8; rest -> zero row N
        flat_tail = np.full(TAIL_NI, N, dtype=np.int16)
        flat_tail[: S * LT_H] = safe_h[FULL * P : FULL * P + LT_H].T.reshape(
            S * LT_H
        )
        flat = np.concatenate([flat_full, flat_tail])
        # SWDGE idx wrap: column j, row p16 = flat[j*16 + p16]
        idx16 = flat.reshape(IDX_COLS, 16).T.astype(np.int16)
        idx_full = np.ascontiguousarray(np.tile(idx16, (8, 1)))    # 8 Q7 groups
        invc_arr = np.ascontiguousarray(inv_pad.reshape(TILES, P).T)  # [P, TILES]
        cnt_row = np.ascontiguousarray(
            cnt_pad.reshape(1, PADT).astype(ml_dtypes.bfloat16)
        )
        half_arrays.append((idx_full, invc_arr, cnt_row))

    in_maps = []
    for c in range(N_CORES):
        bi, hh = c // 2, c % 2
        idx_full, invc_arr, cnt_row = half_arrays[hh]
        in_maps.append(
            {
                "vtx": vtx_pads[bi],
                "wmat": W16,
                "brow": b16,
                "cnt": cnt_row,
                "idx": idx_full,
                "invc": invc_arr,
            }
        )
    return in_maps


def assemble_output(results):
    out = np.empty((B, T, H), dtype=np.float32)
    for c in range(N_CORES):
        bi, hh = c // 2, c % 2
        out[bi, hh * T_CORE : (hh + 1) * T_CORE] = (
            np.asarray(results[c]["out"][:T_CORE]).astype(np.float32)
        )
    return out


_CACHE = {}


def kernel(vertex_feats, hex_to_vertex, W, b):
    include_bias = bool(np.any(np.asarray(b)))
    nc = _CACHE.get(include_bias)
    if nc is None:
        nc = build_module(include_bias=include_bias)
        _CACHE[include_bias] = nc
        _CACHE["nc"] = nc
    in_maps = prep_inputs(vertex_feats, hex_to_vertex, W, b)
    res = run_bass_kernel_spmd(nc, in_maps, list(range(N_CORES)))
    return assemble_output(res.results)


if __name__ == "__main__":
    rng = np.random.default_rng(0)
    vf = rng.standard_normal((B, N, D), dtype=np.float32)
    h2v = rng.integers(-1, N, size=(T, S), dtype=np.int64)
    W = (rng.standard_normal((D, H)) / np.sqrt(D)).astype(np.float32)
    b = np.zeros(H, dtype=np.float32)
    out = kernel(vertex_feats=vf, hex_to_vertex=h2v, W=W, b=b)
    print("out", out.shape, out.dtype, float(np.abs(out).max()))


# revision 7
# speedup vs baseline: 1.6543x; 1.6543x over previous
"""DeflateVertexToHex Trainium2 kernel (bf16 transpose-gather pipeline).

out[b, t, :] = (mean over valid s of vertex_feats[b, hex_to_vertex[t, s], :]) @ W + b

Shapes: vertex_feats [4, 20000, 512] f32, hex_to_vertex [10000, 6] i64,
W [512, 512] f32, b [512] f32 -> out [4, 10000, 512] f32.

Sharding over 8 NeuronCores: core c handles batch c//2 and hex half c%2
(5000 hexes = 39 full 128-hex tiles + one 8-hex tail tile).

The kernel is HBM-gather bound, so all gathered bytes are bf16 (rel err
~4e-3 on the real inputs, well under the 2e-2 gate):
  1. The vertex table is converted host-side to bf16 [N+1, D] with an
     appended all-zero row that padded (-1) adjacency slots remap to.
  2. SWDGE dma_gather in TRANSPOSE mode pulls one tile's rows per
     instruction (768 x 1KB; the ucode rejects transpose gathers
     somewhere between 768 and 1536 idx) and lands them as columns:
     out[p, j, i] = row_i[j*128 + p], i.e. the [D, hex*slot] layout the
     matmul wants. This removes the 12 PE transposes + PSUM round-trip
     per tile that a row-major gather needs.
  3. DVE pools the 6 slots with 3 bf16 adds per tile (2-byte packed
     operands run in the 2x DVE perf mode).
  4. PE accumulates out[hex, H] over 4 K-chunk bf16 matmuls straight
     from the pooled SBUF tile (lhsT = pooled[:, c, :]). When the bias
     is nonzero an extra K=1 pass adds count[hex] * b so the final
     per-hex 1/count scale distributes over it correctly.
  5. ACT applies the per-hex 1/count scale PSUM->SBUF (per-partition
     scale operand); the 128x512 result is written to DRAM as bf16 and
     upcast host-side.
"""

import numpy as np
import ml_dtypes

import concourse.bacc as bacc
import concourse.tile as tile
from concourse import mybir
from concourse.bass_utils import run_bass_kernel_spmd

F32 = mybir.dt.float32
BF16 = mybir.dt.bfloat16
I16 = mybir.dt.int16

B = 4
N = 20000
D = 512
H = 512
T = 10000
S = 6
P = 128
N_CORES = 8
T_CORE = T // 2                  # 5000 hexes per core
FULL = T_CORE // P               # 39 full 128-hex tiles
LT_H = T_CORE - FULL * P         # 8 hexes in the tail tile
TILES = FULL + 1                 # 40
PADT = TILES * P                 # 5120
GROUP = 1                        # full tiles per dma_gather instruction
                                 # (transpose-mode SWDGE fails on HW between
                                 # 768 and 1536 idx; 768 = 1 tile works)
NGROUP = FULL // GROUP           # 13
NI = GROUP * S * P               # 2304 indices per grouped gather
IDXW = S * P // 16               # 48 idx columns per full tile
TAIL_NI = 128                    # tail gather indices (48 real, s*8+h)
IDX_COLS = FULL * IDXW + TAIL_NI // 16  # 1880
JD = D // P                      # 4 K-chunks


def _patch_tile_drain():
    """This container's walrus rejects >1 sync wait on the tail InstDrain
    emitted by TileContext; split the waits across single-wait SP nops."""
    if getattr(tile.TileContext, "_drain_patch_applied", False):
        return

    def _drain_and_barrier_split(self, tick_clock, wait_clock):
        nc = self.nc
        probe = nc.sync.nop(nofuse=True)
        wait_clock.add_sem_waits(
            probe.ins, tile.ScopedClock({None: tick_clock.global_clock})
        )
        si = probe.ins.sync_info
        waits = list(si.on_wait) if si is not None else []
        if si is not None:
            si.on_wait = []
        for w in waits[1:]:
            n = nc.sync.nop(nofuse=True)
            n.ins.sync_info = mybir.SyncInfo(on_wait=[w], on_update=[])
        if waits:
            probe.ins.sync_info = mybir.SyncInfo(on_wait=[waits[0]], on_update=[])
        nc.sync.drain()
        nc.all_engine_barrier()
        assert self.sems is not None
        popped = nc._tile_sem_poison_stack.pop()
        assert popped is self._sem_poison
        nc.clear_and_free_semaphores(list(self.sems.allocated().values()))
        nc.all_engine_barrier()

    tile.TileContext._drain_and_barrier = _drain_and_barrier_split
    tile.TileContext._drain_patch_applied = True


def build_module(include_bias=True, repeat=1, nq=4, scratch=65536, gbufs=3, tbufs=3):
    _patch_tile_drain()
    nc = bacc.Bacc(
        "TRN2",
        target_bir_lowering=False,
        debug=False,
        num_swdge_queues=nq,
        dynamic_dma_scratch_size=scratch,
    )
    vtx = nc.declare_dram_parameter("vtx", [N + 1, D], BF16, isOutput=False)
    wm = nc.declare_dram_parameter("wmat", [D, H], BF16, isOutput=False)
    brow = nc.declare_dram_parameter("brow", [1, H], BF16, isOutput=False)
    cnt = nc.declare_dram_parameter("cnt", [1, PADT], BF16, isOutput=False)
    idx = nc.declare_dram_parameter("idx", [P, IDX_COLS], I16, isOutput=False)
    invc = nc.declare_dram_parameter("invc", [P, TILES], F32, isOutput=False)
    out = nc.declare_dram_parameter("out", [PADT, H], BF16, isOutput=True)

    with tile.TileContext(nc) as tc:
        with (
            tc.tile_pool(name="const", bufs=1) as constp,
            tc.tile_pool(name="gather", bufs=gbufs) as gpool,
            tc.tile_pool(name="tmp", bufs=tbufs) as tmp,
            tc.tile_pool(name="pl", bufs=3) as plp,
            tc.tile_pool(name="osb", bufs=4) as osb,
            tc.tile_pool(name="mmps", bufs=3, space="PSUM") as mmps,
        ):
            # idx first: group 0's slice in its own small DMA so gather 0
            # isn't held behind the full index load.
            idx_sb = constp.tile([P, IDX_COLS], I16)
            c0 = GROUP * IDXW
            nc.sync.dma_start(idx_sb[:, :c0], idx[:, :c0])
            nc.sync.dma_start(idx_sb[:, c0:], idx[:, c0:])
            w_sb = constp.tile([P, JD * H], BF16)
            for c in range(JD):
                nc.sync.dma_start(
                    w_sb[:, c * H : (c + 1) * H], wm[c * P : (c + 1) * P, :]
                )
            invc_sb = constp.tile([P, TILES], F32)
            nc.sync.dma_start(invc_sb[:], invc[:])
            b_sb = constp.tile([1, H], BF16)
            nc.sync.dma_start(b_sb[:], brow[:])
            cnt_sb = constp.tile([1, PADT], BF16)
            nc.sync.dma_start(cnt_sb[:], cnt[:])

            import contextlib

            loop_ctx = tc.For_i(0, repeat, 1) if repeat > 1 else contextlib.nullcontext()
            with loop_ctx:
                for g in range(NGROUP + 1):
                    is_tail = g == NGROUP
                    ni = TAIL_NI if is_tail else NI
                    gt = gpool.tile([P, JD, ni], BF16, tag="gtail" if is_tail else "g")
                    col = g * GROUP * IDXW
                    nc.gpsimd.dma_gather(
                        gt[:],
                        vtx[:],
                        idx_sb[:, col : col + ni // 16],
                        ni,
                        ni,
                        D,
                        transpose=True,
                        queue_num=g % nq,
                    )
                    for k in range(1 if is_tail else GROUP):
                        t = g * GROUP + k
                        hexes = LT_H if is_tail else P
                        w3 = 3 * hexes          # three-slot block width
                        base = k * S * P
                        # slot pool: 2 x (s, s+3) pairs then fold 3 -> 1
                        q = tmp.tile([P, JD, w3], BF16, tag="q")
                        nc.vector.tensor_add(
                            q[:],
                            gt[:, :, base : base + w3],
                            gt[:, :, base + w3 : base + 2 * w3],
                        )
                        r = tmp.tile([P, JD, hexes], BF16, tag="r")
                        nc.vector.tensor_add(
                            r[:], q[:, :, 0:hexes], q[:, :, hexes : 2 * hexes]
                        )
                        pl = plp.tile([P, JD, hexes], BF16, tag="p")
                        nc.vector.tensor_add(pl[:], r[:], q[:, :, 2 * hexes : w3])
                        mmp = mmps.tile([P, H], F32)
                        for c in range(JD):
                            nc.tensor.matmul(
                                mmp[:hexes, :],
                                lhsT=pl[:, c, :],
                                rhs=w_sb[:, c * H : (c + 1) * H],
                                start=(c == 0),
                                stop=(c == JD - 1 and not include_bias),
                            )
                        if include_bias:
                            nc.tensor.matmul(
                                mmp[:hexes, :],
                                lhsT=cnt_sb[0:1, t * P : t * P + hexes],
                                rhs=b_sb[0:1, :],
                                start=False,
                                stop=True,
                            )
                        o = osb.tile([P, H], BF16, tag="o")
                        nc.scalar.mul(
                            o[:hexes, :], mmp[:hexes, :], invc_sb[:hexes, t : t + 1]
                        )
                        nc.sync.dma_start(out[t * P : t * P + hexes, :], o[:hexes, :])
    nc.finalize()
    return nc


def prep_inputs(vertex_feats, hex_to_vertex, W, b):
    """Host-side prep -> per-core in_maps."""
    vertex_feats = np.asarray(vertex_feats)
    hex_to_vertex = np.asarray(hex_to_vertex)
    W16 = np.ascontiguousarray(np.asarray(W).astype(ml_dtypes.bfloat16))
    b16 = np.asarray(b).astype(ml_dtypes.bfloat16).reshape(1, H)

    mask = hex_to_vertex >= 0
    safe = np.where(mask, hex_to_vertex, N).astype(np.int16)       # [T, 6]
    cntc = np.maximum(mask.sum(axis=1), 1).astype(np.float32)      # [T]
    inv = (1.0 / cntc).astype(np.float32)

    vtx_pads = []
    for bi in range(B):
        vp = np.zeros((N + 1, D), dtype=ml_dtypes.bfloat16)
        vp[:N] = vertex_feats[bi].astype(ml_dtypes.bfloat16)
        vtx_pads.append(vp)

    half_arrays = []
    for hh in range(2):
        sl = slice(hh * T_CORE, (hh + 1) * T_CORE)
        safe_h = safe[sl]                                          # [5000, 6]
        cnt_pad = np.ones(PADT, dtype=np.float32)
        cnt_pad[:T_CORE] = cntc[sl]
        inv_pad = np.ones(PADT, dtype=np.float32)
        inv_pad[:T_CORE] = inv[sl]
        # full tiles: flat[t, s*128 + p] = safe_h[t*128 + p, s]
        flat_full = (
            safe_h[: FULL * P]
            .reshape(FULL, P, S)
            .transpose(0, 2, 1)
            .reshape(FULL * S * P)
        )
        # tail: i = s*8 + h for s < 6, h < 8; rest -> zero row N
        flat_tail = np.full(TAIL_NI, N, dtype=np.int16)
        flat_tail[: S * LT_H] = safe_h[FULL * P : FULL * P + LT_H].T.reshape(
            S * LT_H
        )
        flat = np.concatenate([flat_full, flat_tail])
        # SWDGE idx wrap: column j, row p16 = flat[j*16 + p16]
        idx16 = flat.reshape(IDX_COLS, 16).T.astype(np.int16)
        idx_full = np.ascontiguousarray(np.tile(idx16, (8, 1)))    # 8 Q7 groups
        invc_arr = np.ascontiguousarray(inv_pad.reshape(TILES, P).T)  # [P, TILES]
        cnt_row = np.ascontiguousarray(
            cnt_pad.reshape(1, PADT).astype(ml_dtypes.bfloat16)
        )
        half_arrays.append((idx_full, invc_arr, cnt_row))

    in_maps = []
    for c in range(N_CORES):
        bi, hh = c // 2, c % 2
        idx_full, invc_arr, cnt_row = half_arrays[hh]
        in_maps.append(
            {
                "vtx": vtx_pads[bi],
                "wmat": W16,
                "brow": b16,
                "cnt": cnt_row,
                "idx": idx_full,
                "invc": invc_arr,
            }
        )
    return in_maps


def assemble_output(results):
    out = np.empty((B, T, H), dtype=np.float32)
    for c in range(N_CORES):
        bi, hh = c // 2, c % 2
        out[bi, hh * T_CORE : (hh + 1) * T_CORE] = (
            np.asarray(results[c]["out"][:T_CORE]).astype(np.float32)
        )
    return out


_CACHE = {}


def kernel(vertex_feats, hex_to_vertex, W, b):
    include_bias = bool(np.any(np.asarray(b)))
    nc = _CACHE.get(include_bias)
    if nc is None:
        nc = build_module(include_bias=include_bias)
        _CACHE[include_bias] = nc
        _CACHE["nc"] = nc
    in_maps = prep_inputs(vertex_feats, hex_to_vertex, W, b)
    res = run_bass_kernel_spmd(nc, in_maps, list(range(N_CORES)))
    return assemble_output(res.results)


if __name__ == "__main__":
    rng = np.random.default_rng(0)
    vf = rng.standard_normal((B, N, D), dtype=np.float32)
    h2v = rng.integers(-1, N, size=(T, S), dtype=np.int64)
    W = (rng.standard_normal((D, H)) / np.sqrt(D)).astype(np.float32)
    b = np.zeros(H, dtype=np.float32)
    out = kernel(vertex_feats=vf, hex_to_vertex=h2v, W=W, b=b)
    print("out", out.shape, out.dtype, float(np.abs(out).max()))
